# revision 20
# baseline (speedup 1.0000x reference)
"""Trainium2 Bass kernel for nn_Attention_54142357733562 (linear attention).

Reference math (per batch b, x flattened to [C, N]):
    Q = wq @ x ; K = wk @ x ; V = wv @ x
    Qn = Q / ||Q||_c ; Kn = K / ||K||_c
    k_sum = sum_n Kn + EPS
    out = (value_sum + kv^T Qn) / (N + Qn^T k_sum),  kv = Kn V^T

Algebraic restructure used here (all matmul inputs bf16):
    s[n] = ||Q[:, n]||;  den[n] = N*s[n] + Q[:, n].k_sum
    G'   = [Kn | 1]^T [x^T | 1]        # [33, 258]: G = Kn x^T, ksum, x_sum, n
    (AllGather G' over the 2-core pair + local add == AllReduce, but cheaper)
    kvp  = G'[:, 0:256] @ wv^T          # rows 0:32 = kv, row 32 = value_sum
    out[c, n] = sum_m [kvp][m, c] * ([Q; s][m, n] / den[n])
V is never materialized: kv == (Kn x^T) wv^T and value_sum == wv x_sum,
so phase 1 only computes Q, K (64 channels) plus a PE transpose of x.

Sharding: 8 cores = 4 batches x 2 N-halves. x is uploaded in bf16 (16 MiB
per core), out is written in bf16 and upcast on the host; HBM traffic is
half of the f32 version. The AllReduce payload is [33, 258] f32.
"""

import numpy as np
from contextlib import ExitStack

import concourse.bass as bass
import concourse.mybir as mybir
import concourse.tile as tile
from concourse import bacc
from concourse.bass_utils import run_bass_kernel_spmd
from concourse.masks import make_identity

F32 = mybir.dt.float32
BF16 = mybir.dt.bfloat16

C = 256
CQK = 32
P = 128
NT = 1024  # macro-tile width along N
ST = NT // P  # 8
SW = 66    # stash row: [Q 0:32 | s 32 | ||K|| 33 | K 34:66]
GW = 291   # g_in row: [Kn 0:32 | 1 | x^T 33:289 | 1 | 1]
EPS = 1e-6


def emit_attention(tc, xs, wqk, wvt, out, nsh, n_total, groups,
                   use_collective=True, phases=(1, 2)):
    nc = tc.nc
    NM = nsh // NT
    SROW = nsh // P

    xs_r = xs.rearrange("(o p) n -> p o n", p=P)    # [128, 2, nsh]
    out_r = out.rearrange("(o p) n -> p o n", p=P)
    wqk_r = wqk.rearrange("(o p) j -> p o j", p=P)  # [128, 2, 64]
    wvt_r = wvt.rearrange("(o p) c -> p o c", p=P)  # [128, 2, 256]

    mult = mybir.AluOpType.mult

    with ExitStack() as ctx:
        singles = ctx.enter_context(tc.tile_pool(name="singles", bufs=1))
        dram = ctx.enter_context(tc.tile_pool(name="dram", bufs=1, space="DRAM"))

        wqk_sb = singles.tile([P, 2, 2 * CQK], BF16)
        nc.sync.dma_start(wqk_sb, wqk_r)
        wvt_sb = singles.tile([P, 2, C], BF16)
        nc.sync.dma_start(wvt_sb, wvt_r)
        ident_f = singles.tile([P, P], F32)
        make_identity(nc, ident_f)
        ident_b = singles.tile([P, P], BF16)
        nc.vector.tensor_copy(ident_b, ident_f)

        stash = singles.tile([P, SROW, SW], BF16)

        # ---------------- phase 1: QK + x^T + G accumulation ----------------
        if 1 in phases:
            with ExitStack() as p1:
                xp = p1.enter_context(tc.tile_pool(name="xp", bufs=4))
                gi = p1.enter_context(tc.tile_pool(name="gi", bufs=4))
                scr = p1.enter_context(tc.tile_pool(name="scr", bufs=4))
                ps_qk = p1.enter_context(
                    tc.tile_pool(name="ps_qk", bufs=2, space="PSUM"))
                ps_xt = p1.enter_context(
                    tc.tile_pool(name="ps_xt", bufs=5, space="PSUM"))
                ps_g = p1.enter_context(
                    tc.tile_pool(name="ps_g", bufs=1, space="PSUM"))

                g_acc = ps_g.tile([P, 512], F32)  # rows 0:33, cols 0:258 used

                xdrain = 0
                for m in range(NM):
                    xt = xp.tile([P, 2, NT], BF16)
                    nc.sync.dma_start(xt, xs_r[:, :, m * NT:(m + 1) * NT])

                    gi_t = gi.tile([P, ST, GW], BF16)
                    nc.gpsimd.memset(gi_t[:, :, CQK:CQK + 1], 1.0)
                    nc.gpsimd.memset(gi_t[:, :, GW - 2:GW], 1.0)

                    qk_ps = ps_qk.tile([P, ST, 64], F32)  # 1 bank
                    for h in range(ST // 2):
                        xt_ps = ps_xt.tile([P, 2, 512], BF16)  # 1 bank
                        for s2 in range(2):
                            s = 2 * h + s2
                            ch = slice(s * P, (s + 1) * P)
                            for o in range(2):
                                nc.tensor.matmul(
                                    qk_ps[:, s, :],
                                    xt[:, o, ch],
                                    wqk_sb[:, o, :],
                                    start=(o == 0),
                                    stop=(o == 1),
                                )
                            for o in range(2):
                                nc.tensor.transpose(
                                    xt_ps[:, s2, o * P:(o + 1) * P],
                                    xt[:, o, ch],
                                    ident_b,
                                )
                        # x^T drain (bf16 -> bf16; 2x on DVE)
                        xt_dst = gi_t[:, 2 * h:2 * h + 2, CQK + 1:CQK + 1 + C]
                        if xdrain % 8 < 5:
                            nc.vector.tensor_copy(xt_dst, xt_ps[:, :, 0:C])
                        else:
                            nc.scalar.copy(xt_dst, xt_ps[:, :, 0:C])
                        xdrain += 1

                    # QK drain once per macro (strided into stash) on ACT
                    mst = stash[:, m * ST:(m + 1) * ST, :]
                    qk_dst = bass.AP(
                        tensor=mst.tensor,
                        offset=mst.offset,
                        ap=[mst.ap[0], mst.ap[1], [CQK + 2, 2], [1, CQK]],
                    )
                    nc.scalar.copy(qk_dst, qk_ps.rearrange("p h (g c) -> p h g c", g=2))

                    # norms: s = ||Q||, ||K||; Kn = K / ||K||
                    qk_view = bass.AP(
                        tensor=mst.tensor,
                        offset=mst.offset,
                        ap=[mst.ap[0], mst.ap[1], [CQK + 2, 2], [1, CQK]],
                    )
                    sq = scr.tile([P, ST, 2, CQK], BF16, tag="sq")
                    nc.vector.tensor_tensor(sq, qk_view, qk_view, mult)
                    ssq = scr.tile([P, ST, 2], BF16, tag="ssq")
                    with nc.allow_low_precision(reason="bf16 norm sums"):
                        nc.vector.reduce_sum(ssq, sq, axis=mybir.AxisListType.X)
                    nc.scalar.sqrt(mst[:, :, CQK:CQK + 2], ssq)
                    rk = scr.tile([P, ST, 1], F32, tag="rk")
                    nc.vector.reciprocal(rk, mst[:, :, CQK + 1:CQK + 2])
                    nc.gpsimd.tensor_tensor(gi_t[:, :, 0:CQK],
                                            mst[:, :, CQK + 2:SW],
                                            rk.to_broadcast((P, ST, CQK)), mult)

                    for s in range(ST):
                        nc.tensor.matmul(
                            g_acc[0:CQK + 1, 0:C + 2],
                            gi_t[:, s, 0:CQK + 1],
                            gi_t[:, s, CQK + 1:GW],
                            start=(m == 0 and s == 0),
                            stop=(m == NM - 1 and s == ST - 1),
                        )

                g_sb = singles.tile([CQK + 1, C + 2], F32)
                nc.vector.tensor_copy(g_sb, g_acc[0:CQK + 1, 0:C + 2])

        cc_in = dram.tile([CQK + 1, C + 2], F32)
        cc_out = dram.tile([2, CQK + 1, C + 2], F32)
        if 1 in phases:
            nc.sync.dma_start(cc_in, g_sb)
        if use_collective and 1 in phases:
            nc.gpsimd.collective_compute(
                "AllGather",
                mybir.AluOpType.bypass,
                replica_groups=groups,
                ins=[cc_in.opt()],
                outs=[cc_out.opt()],
            )
        elif 1 in phases:
            for k in range(2):
                nc.sync.dma_start(cc_out[k], cc_in)

        if 2 not in phases:
            # keep phase 1 live: route the collective result to 'out'
            res_sb = singles.tile([CQK + 1, C + 2], F32)
            nc.sync.dma_start(res_sb, cc_out[0])
            res_b = singles.tile([CQK + 1, C + 2], BF16)
            nc.vector.tensor_copy(res_b, res_sb)
            nc.sync.dma_start(out_r[0:CQK + 1, 0, 0:C + 2], res_b)
            return

        # ------------- post-collective: ksum vector first, kvp = G wv^T -------------
        ksum_f = singles.tile([P, 2, CQK], F32)
        for k in range(2):
            nc.sync.dma_start(ksum_f[:, k, :],
                              cc_out[k, 0:CQK, C:C + 1].partition_broadcast(P))
        ksum_s = singles.tile([P, CQK], F32)
        nc.vector.tensor_tensor(ksum_s, ksum_f[:, 0, :], ksum_f[:, 1, :],
                                mybir.AluOpType.add)
        ksum = singles.tile([P, CQK + 2], BF16)
        nc.vector.tensor_scalar_add(ksum[:, 0:CQK], ksum_s, EPS)
        nc.vector.memset(ksum[:, CQK:CQK + 1], float(n_total))
        gsb2 = singles.tile([CQK + 1, 2, C + 2], F32)
        nc.sync.dma_start(gsb2, cc_out.rearrange("k m c -> m k c"))
        gsb = singles.tile([CQK + 1, C + 2], F32)
        nc.vector.tensor_tensor(gsb, gsb2[:, 0, :], gsb2[:, 1, :],
                                mybir.AluOpType.add)

        gT_sb = singles.tile([P, 2, CQK + 2], BF16)
        kvp_sb = singles.tile([CQK + 1, C], BF16)
        with ExitStack() as pk:
            ps_k = pk.enter_context(tc.tile_pool(name="ps_k", bufs=2,
                                                 space="PSUM"))
            gT_ps = ps_k.tile([P, 2, CQK + 1], F32)
            for k in range(2):
                nc.tensor.transpose(gT_ps[:, k, :],
                                    gsb[0:CQK + 1, k * P:(k + 1) * P],
                                    ident_f[0:CQK + 1, 0:CQK + 1])
            nc.vector.tensor_copy(gT_sb[:, :, 0:CQK + 1], gT_ps)
            kvp_ps = ps_k.tile([CQK + 1, C], F32)
            for k in range(2):
                nc.tensor.matmul(
                    kvp_ps,
                    gT_sb[:, k, 0:CQK + 1],
                    wvt_sb[:, k, :],
                    start=(k == 0),
                    stop=(k == 1),
                )
            nc.vector.tensor_copy(kvp_sb, kvp_ps)

        # ---------------- phase 2: out = kvp^T ([Q; s] / den) ----------------
        with ExitStack() as p2:
            scr2 = p2.enter_context(tc.tile_pool(name="scr2", bufs=4))
            qtp = p2.enter_context(tc.tile_pool(name="qtp", bufs=4))
            outp = p2.enter_context(tc.tile_pool(name="outp", bufs=5))
            ps_qt = p2.enter_context(tc.tile_pool(name="ps_qt", bufs=2,
                                                  space="PSUM"))
            ps_out = p2.enter_context(tc.tile_pool(name="ps_out", bufs=6,
                                                   space="PSUM"))

            MP = ST  # one macro: [128, 8, .] rows, 1024 n

            def emit_prod(m):
                st_sl = stash[:, m * MP:(m + 1) * MP, 0:CQK + 1]
                prod = scr2.tile([P, MP, CQK + 1], BF16, tag="prod")
                nc.gpsimd.tensor_tensor(
                    prod, st_sl,
                    ksum[:, None, 0:CQK + 1].to_broadcast((P, MP, CQK + 1)),
                    mult)
                return prod

            for m in range(NM):
                st_sl = stash[:, m * MP:(m + 1) * MP, 0:CQK + 1]

                prod = emit_prod(m)
                den = scr2.tile([P, MP, 1], BF16, tag="den")
                with nc.allow_low_precision(reason="bf16 den sum"):
                    nc.vector.reduce_sum(den, prod, axis=mybir.AxisListType.X)
                d = scr2.tile([P, MP, 1], BF16, tag="d")
                with nc.allow_low_precision(reason="bf16 reciprocal"):
                    nc.vector.reciprocal(d, den)
                qsc = scr2.tile([P, MP, CQK + 2], BF16, tag="qsc")
                nc.gpsimd.tensor_tensor(
                    qsc[:, :, 0:CQK + 1], st_sl,
                    d.to_broadcast((P, MP, CQK + 1)), mult)

                qt_ps = ps_qt.tile([CQK + 1, MP, P], BF16)  # 1 bank
                for s in range(MP):
                    nc.tensor.transpose(qt_ps[:, s, :], qsc[:, s, 0:CQK + 1],
                                        ident_b)
                qt_sb = qtp.tile([CQK + 1, MP * P], BF16)
                nc.vector.tensor_copy(qt_sb, qt_ps)

                ot = outp.tile([P, 2, NT], BF16)
                for mh in range(2):
                    for blk in range(2):
                        o_ps = ps_out.tile([P, NT // 2], F32, tag="o_ps")
                        nc.tensor.matmul(
                            o_ps,
                            kvp_sb[:, blk * P:(blk + 1) * P],
                            qt_sb[:, mh * (NT // 2):(mh + 1) * (NT // 2)],
                            start=True,
                            stop=True,
                        )
                        dst = ot[:, blk, mh * (NT // 2):(mh + 1) * (NT // 2)]
                        if (mh, blk) == (0, 0):
                            nc.vector.tensor_copy(dst, o_ps)
                        else:
                            nc.scalar.copy(dst, o_ps)
                nc.sync.dma_start(out_r[:, :, m * NT:(m + 1) * NT], ot)


def build_attention_nc(nsh, n_total, num_cores, groups, repeat=1,
                       use_collective=True, phases=(1, 2)):
    nc = bacc.Bacc("TRN2", target_bir_lowering=False, debug=False,
                   num_devices=num_cores)
    xs = nc.dram_tensor("xs", [C, nsh], BF16, kind="ExternalInput").ap()
    wqk = nc.dram_tensor("wqk", [C, 2 * CQK], BF16, kind="ExternalInput").ap()
    wvt = nc.dram_tensor("wvt", [C, C], BF16, kind="ExternalInput").ap()
    out = nc.dram_tensor("out", [C, nsh], BF16, kind="ExternalOutput").ap()
    with tile.TileContext(nc) as tc:
        for _ in range(repeat):
            emit_attention(tc, xs, wqk, wvt, out, nsh, n_total, groups,
                           use_collective=use_collective, phases=phases)
    nc.compile()
    return nc


_NC_CACHE = {}


def _get_nc(nsh, n_total, num_cores, groups_key):
    key = (nsh, n_total, num_cores, groups_key)
    if key not in _NC_CACHE:
        groups = [list(g) for g in groups_key]
        _NC_CACHE[key] = build_attention_nc(nsh, n_total, num_cores, groups)
    return _NC_CACHE[key]


def make_in_maps(inputs, nsh):
    """Host-side prep: bf16 casts + per-core shards. inputs: full arrays."""
    import ml_dtypes
    bf = ml_dtypes.bfloat16
    x = np.asarray(inputs["x"])
    B, Cc, H, W = x.shape
    N = H * W
    spb = N // nsh  # shards per batch
    xr = x.reshape(B, Cc, N)
    wqk = np.ascontiguousarray(
        np.concatenate([np.asarray(inputs["wq"]).T,
                        np.asarray(inputs["wk"]).T], axis=1)).astype(bf)
    wvt = np.ascontiguousarray(np.asarray(inputs["wv"]).T).astype(bf)
    in_maps = []
    for core in range(B * spb):
        b, hh = core // spb, core % spb
        in_maps.append({
            "xs": np.ascontiguousarray(
                xr[b, :, hh * nsh:(hh + 1) * nsh]).astype(bf),
            "wqk": wqk,
            "wvt": wvt,
        })
    return in_maps


def _kernel_numpy(x, wq, bq, wk, bk, wv, bv):
    b, c, h, w = x.shape
    n = h * w
    xf = x.reshape(b, c, n).astype(np.float64)
    Q = np.einsum("oc,bcn->bon", wq.astype(np.float64), xf) + bq.astype(np.float64)[None, :, None]
    K = np.einsum("oc,bcn->bon", wk.astype(np.float64), xf) + bk.astype(np.float64)[None, :, None]
    V = np.einsum("oc,bcn->bon", wv.astype(np.float64), xf) + bv.astype(np.float64)[None, :, None]
    Qn = Q / np.linalg.norm(Q, axis=1, keepdims=True)
    Kn = K / np.linalg.norm(K, axis=1, keepdims=True)
    k_sum = Kn.sum(-1) + EPS
    tailor = 1.0 / (n + np.einsum("bmn,bm->bn", Qn, k_sum))
    value_sum = V.sum(-1)
    kv = np.einsum("bmn,bcn->bmc", Kn, V)
    ms = value_sum[:, :, None] + np.einsum("bmn,bmc->bcn", Qn, kv)
    return (ms * tailor[:, None, :]).reshape(b, c, h, w).astype(np.float32)


def kernel(x, wq, bq, wk, bk, wv, bv):
    x = np.asarray(x, dtype=np.float32)
    B, Cc, H, W = x.shape
    if (any(np.any(np.asarray(b_) != 0) for b_ in (bq, bk, bv))
            or Cc != C or wq.shape != (CQK, C) or wv.shape != (C, C)
            or (H * W) % (2 * NT) != 0 or B != 4):
        return _kernel_numpy(x, wq, bq, wk, bk, wv, bv)
    N = H * W
    ncores = 8
    spb = ncores // B  # 2
    nsh = N // spb
    groups_key = tuple(
        tuple(range(b * spb, (b + 1) * spb)) for b in range(B))

    nc = _get_nc(nsh, N, ncores, groups_key)
    in_maps = make_in_maps(
        {"x": x, "wq": wq, "wk": wk, "wv": wv}, nsh)
    res = run_bass_kernel_spmd(nc, in_maps, list(range(ncores)))

    out = np.empty((B, Cc, N), np.float32)
    for core in range(ncores):
        b, hh = core // spb, core % spb
        out[b, :, hh * nsh:(hh + 1) * nsh] = \
            np.asarray(res.results[core]["out"]).astype(np.float32)
    return out.reshape(B, Cc, H, W)
